# revision 5
# baseline (speedup 1.0000x reference)
"""ArcFace loss (B=8192, D=512, C=500000) on 8 TRN2 NeuronCores.

v4 strategy - the device kernel is reduced to the one irreducible piece of
work: the B x B cosine matmul and the row-wise sum of exp.  Everything
else (per-row scalars, O(B*D) vector math) moves to the host:
  - Host gathers centers = W[labels], L2-normalizes both x and the
    centers, pre-scales by 16 and casts to fp8e4 (the matmul then yields
    256*cos, and the device exp uses the constant scale S/256).
  - Host computes the exact diagonal cosine t_i = xn_i . cn_i in f32, the
    margin term t' = cos(arccos(t)+M), the sum-exp diagonal correction
    exp(S*t') - exp(S*t), and the final label-smoothed loss from the
    device row-sums (including the tiny eps/C * S * sum_j cos'_ij term the
    v3 kernel dropped).
  - Device (row-sharded, core k owns batch rows [k*1024, (k+1)*1024)):
    stream all 8192 normalized centers (fp8, replicated 4MB) against the
    core's own 1024 x-rows (stationary fp8 SwInterleave blocks).  Main
    loop: 4 column chunks x 8 row tiles x [128 x 2048] psum blocks
    (2 chunks ping-pong = all 8 psum banks); ScalarE Exp with accum_out
    produces the row-sums directly.  256 DoubleRowSwInterleave matmuls at
    the PE's streaming rate; LDWEIGHTS fully hidden under the previous
    matmul.  No collective, no device prefix/tail: each core DMAs out its
    [128, 8] partial sum-exp and the host assembles the loss.
"""

import sys

if "/opt/trn_rl_repo" not in sys.path:
    sys.path.insert(0, "/opt/trn_rl_repo")

import math

import numpy as np
import ml_dtypes

import concourse.bacc as bacc
import concourse.tile as tile
from concourse import mybir
from concourse.bass_utils import run_bass_kernel_spmd

F32 = mybir.dt.float32
BF16 = mybir.dt.bfloat16
FP8 = mybir.dt.float8e4
P = 128

# problem constants (hardcoded; kernel.py must be self-contained)
B, D, C = 8192, 512, 500000
NCORES = 8
MARGIN, S_SCALE, EPS = 0.5, 64.0, 0.1
GAM = 16.0                       # fp8 pre-scale on xn and cn
EXP_SCALE = S_SCALE / (GAM * GAM)

BL = B // NCORES                 # 1024 own rows per core
NM = BL // P                     # 8 own row tiles
KC = D // P                      # 4 contraction chunks of 128
NKG = KC // 2                    # 2 double-row passes
NC_CH = 4                        # column chunks per row tile
CW = B // NC_CH                  # 2048 columns per chunk (4 psum banks)
NH = CW // 512                   # 4 matmuls of 512 per (chunk, kg)


def build_nc():
    nc = bacc.Bacc(
        "TRN2",
        target_bir_lowering=False,
        debug=False,
        enable_asserts=False,
        num_devices=NCORES,
    )
    xw_ext = nc.dram_tensor("xw8", [P, NM * NKG * 2 * P], FP8, kind="ExternalInput")
    cn_ext = nc.dram_tensor("cnt8", [P, NC_CH * KC * CW], FP8, kind="ExternalInput")
    out_ext = nc.dram_tensor("sout", [P, NM * NC_CH], F32, kind="ExternalOutput")

    with tile.TileContext(nc) as tc:
        with (
            tc.tile_pool(name="const", bufs=1) as const,
            tc.tile_pool(name="psum", bufs=2, space="PSUM") as psum,
        ):
            xw = const.tile([P, NM * NKG * 2 * P], FP8, name="xw")
            cnt = const.tile([P, NC_CH * KC * CW], FP8, name="cnt")
            seacc = const.tile([P, NM * NC_CH], F32, name="seacc")

            cnt3 = cnt[:].rearrange("p (k n) -> p k n", n=CW)
            cn_ext3 = cn_ext[:, :].rearrange("p (k n) -> p k n", n=CW)

            # lead-in: the stationary tensor rides the scalar queue (a
            # separate DMA ring, and ScalarE is idle until exp #1) while the
            # streamed centers flow on the sync ring, first chunk finely
            # split in exact matmul consumption order.
            nc.scalar.dma_start(out=xw[:, : 2 * 2 * P], in_=xw_ext[:, : 2 * 2 * P])
            nc.scalar.dma_start(out=xw[:, 2 * 2 * P :], in_=xw_ext[:, 2 * 2 * P :])
            nc.sync.dma_start(out=cnt3[:, 0:2, 0:512], in_=cn_ext3[:, 0:2, 0:512])
            nc.sync.dma_start(
                out=cnt3[:, 0:2, 512:1024], in_=cn_ext3[:, 0:2, 512:1024]
            )
            nc.sync.dma_start(
                out=cnt3[:, 0:2, 1024:2048], in_=cn_ext3[:, 0:2, 1024:2048]
            )
            nc.sync.dma_start(
                out=cnt3[:, 2:4, 0:1024], in_=cn_ext3[:, 2:4, 0:1024]
            )
            nc.sync.dma_start(
                out=cnt3[:, 2:4, 1024:2048], in_=cn_ext3[:, 2:4, 1024:2048]
            )
            for c in range(1, NC_CH):
                nc.sync.dma_start(
                    out=cnt3[:, c * KC : (c + 1) * KC, :],
                    in_=cn_ext3[:, c * KC : (c + 1) * KC, :],
                )

            for c in range(NC_CH):
                for m in range(NM):
                    ps = psum.tile([P, CW], F32, name="ps")
                    for kg in range(NKG):
                        wo = (m * NKG + kg) * 2 * P
                        for h in range(NH):
                            nc.tensor.matmul(
                                out=ps[:, h * 512 : (h + 1) * 512],
                                lhsT=xw[:, wo : wo + 2 * P],
                                rhs=cnt3[
                                    :,
                                    c * KC + 2 * kg : c * KC + 2 * kg + 2,
                                    h * 512 : (h + 1) * 512,
                                ],
                                start=(kg == 0),
                                stop=(kg == NKG - 1),
                                perf_mode=mybir.MatmulPerfMode.DoubleRowSwInterleave,
                            )
                    # exp in place (PSUM out has lower access latency than
                    # SBUF and the exp values themselves are dead - only the
                    # accumulator row-sum is used)
                    nc.scalar.activation(
                        out=ps[:],
                        in_=ps[:],
                        func=mybir.ActivationFunctionType.Exp,
                        scale=EXP_SCALE,
                        accum_out=seacc[:, m * NC_CH + c : m * NC_CH + c + 1],
                    )

            nc.sync.dma_start(out=out_ext[:, :], in_=seacc[:])

    nc.compile()
    return nc


def _pack_stationary(xn8_rows):
    """[1024, 512] fp8 -> [128, NM*NKG*256] SwInterleave stationary blocks.

    Block (m, kg) at column offset (m*NKG+kg)*256 holds
    packed[p, 2*(127-r) + i] = xn8[m*128 + r, (2*kg+i)*128 + p].
    """
    a = xn8_rows.reshape(NM, P, NKG, 2, P)          # [m, r, kg, i, p]
    a = a.transpose(4, 0, 2, 1, 3)[:, :, :, ::-1, :]  # [p, m, kg, r(rev), i]
    return np.ascontiguousarray(a.reshape(P, NM * NKG * 2 * P))


def _pack_streaming(cn8):
    """[8192, 512] fp8 -> [128, NC_CH*KC*CW]: piece (c, kk) holds
    cn8.T[kk*128 + p, c*CW + n]."""
    a = cn8.T.reshape(KC, P, NC_CH, CW).transpose(1, 2, 0, 3)  # [p, c, kk, n]
    return np.ascontiguousarray(a.reshape(P, NC_CH * KC * CW))


def prepare(x, labels, W):
    """All host-side math: normalize, pack fp8 inputs, and return the
    per-row constants needed to assemble the loss from device row-sums."""
    x = np.asarray(x, dtype=np.float32)
    W = np.asarray(W, dtype=np.float32)
    labels = np.asarray(labels).astype(np.int64)

    centers = W[labels]                                  # [B, D]
    cn = centers / np.linalg.norm(centers, axis=1, keepdims=True)
    xn = x / np.maximum(np.linalg.norm(x, axis=1, keepdims=True), 1e-12)

    xn8 = (xn * GAM).astype(ml_dtypes.float8_e4m3)
    cn8 = (cn * GAM).astype(ml_dtypes.float8_e4m3)

    cnt = _pack_streaming(cn8)
    in_maps = []
    for k in range(NCORES):
        xw = _pack_stationary(xn8[k * BL : (k + 1) * BL])
        in_maps.append({"xw8": xw, "cnt8": cnt})

    # exact per-row scalars in f64
    xn64 = xn.astype(np.float64)
    cn64 = cn.astype(np.float64)
    t = np.clip(np.sum(xn64 * cn64, axis=1), -1.0, 1.0)
    tp = np.cos(np.arccos(t) + MARGIN)
    ecorr = np.exp(S_SCALE * tp) - np.exp(S_SCALE * t)
    rowlin = xn64 @ cn64.sum(axis=0) + (tp - t)          # sum_j cos'_ij
    return in_maps, t, tp, ecorr, rowlin


_compiled_nc = None


def get_compiled():
    global _compiled_nc
    if _compiled_nc is None:
        _compiled_nc = build_nc()
    return _compiled_nc


def run(x, labels, W, trace=False, trace_cores=None):
    nc = get_compiled()
    in_maps, t, tp, ecorr, rowlin = prepare(x, labels, W)
    res = run_bass_kernel_spmd(
        nc,
        in_maps,
        core_ids=list(range(NCORES)),
        trace=trace,
        trace_cores=trace_cores,
    )
    # sout[p, m*NC_CH + c] holds the partial sum over column chunk c for
    # local row m*128 + p; sum chunks, then flatten [m, p] -> local rows
    rowsum = np.concatenate(
        [
            np.asarray(r["sout"], dtype=np.float64)
            .reshape(P, NM, NC_CH)
            .sum(axis=2)
            .T.reshape(BL)
            for r in res.results
        ]
    )
    lse = np.log(rowsum + ecorr)
    a1 = (1.0 - EPS) + EPS * B / C
    loss = np.mean(
        a1 * lse - (1.0 - EPS) * S_SCALE * tp - (EPS / C) * S_SCALE * rowlin
    )
    return np.float32(loss), res


def kernel(**inputs):
    loss, _ = run(inputs["x"], inputs["labels"], inputs["W"])
    return loss


# revision 7
# speedup vs baseline: 1.0139x; 1.0139x over previous
"""ArcFace loss (B=8192, D=512, C=500000) on 8 TRN2 NeuronCores.

v4 strategy - the device kernel is reduced to the one irreducible piece of
work: the B x B cosine matmul and the row-wise sum of exp.  Everything
else (per-row scalars, O(B*D) vector math) moves to the host:
  - Host gathers centers = W[labels], L2-normalizes both x and the
    centers, pre-scales by 16 and casts to fp8e4 (the matmul then yields
    256*cos, and the device exp uses the constant scale S/256).
  - Host computes the exact diagonal cosine t_i = xn_i . cn_i in f32, the
    margin term t' = cos(arccos(t)+M), the sum-exp diagonal correction
    exp(S*t') - exp(S*t), and the final label-smoothed loss from the
    device row-sums (including the tiny eps/C * S * sum_j cos'_ij term the
    v3 kernel dropped).
  - Device (row-sharded, core k owns batch rows [k*1024, (k+1)*1024)):
    stream all 8192 normalized centers (fp8, replicated 4MB) against the
    core's own 1024 x-rows (stationary fp8 SwInterleave blocks).  Main
    loop: 4 column chunks x 8 row tiles x [128 x 2048] psum blocks
    (2 chunks ping-pong = all 8 psum banks); ScalarE Exp with accum_out
    produces the row-sums directly.  256 DoubleRowSwInterleave matmuls at
    the PE's streaming rate; LDWEIGHTS fully hidden under the previous
    matmul.  No collective, no device prefix/tail: each core DMAs out its
    [128, 8] partial sum-exp and the host assembles the loss.
"""

import sys

if "/opt/trn_rl_repo" not in sys.path:
    sys.path.insert(0, "/opt/trn_rl_repo")

import math

import numpy as np
import ml_dtypes

import concourse.bacc as bacc
import concourse.tile as tile
from concourse import mybir
from concourse.bass_utils import run_bass_kernel_spmd

F32 = mybir.dt.float32
BF16 = mybir.dt.bfloat16
FP8 = mybir.dt.float8e4
P = 128

# problem constants (hardcoded; kernel.py must be self-contained)
B, D, C = 8192, 512, 500000
NCORES = 8
MARGIN, S_SCALE, EPS = 0.5, 64.0, 0.1
GAM = 16.0                       # fp8 pre-scale on xn and cn
EXP_SCALE = S_SCALE / (GAM * GAM)

BL = B // NCORES                 # 1024 own rows per core
NM = BL // P                     # 8 own row tiles
KC = D // P                      # 4 contraction chunks of 128
NKG = KC // 2                    # 2 double-row passes
NC_CH = 4                        # column chunks per row tile
CW = B // NC_CH                  # 2048 columns per chunk (4 psum banks)
NH = CW // 512                   # 4 matmuls of 512 per (chunk, kg)


def build_nc():
    nc = bacc.Bacc(
        "TRN2",
        target_bir_lowering=False,
        debug=False,
        enable_asserts=False,
        num_devices=NCORES,
    )
    xw_ext = nc.dram_tensor("xw8", [P, NM * NKG * 2 * P], FP8, kind="ExternalInput")
    cn_ext = nc.dram_tensor("cnt8", [P, NC_CH * KC * CW], FP8, kind="ExternalInput")
    out_ext = nc.dram_tensor("sout", [P, NM * NC_CH], F32, kind="ExternalOutput")

    with tile.TileContext(nc) as tc:
        with (
            tc.tile_pool(name="const", bufs=1) as const,
            tc.tile_pool(name="psum", bufs=2, space="PSUM") as psum,
        ):
            xw = const.tile([P, NM * NKG * 2 * P], FP8, name="xw")
            cnt = const.tile([P, NC_CH * KC * CW], FP8, name="cnt")
            seacc = const.tile([P, NM * NC_CH], F32, name="seacc")

            cnt3 = cnt[:].rearrange("p (k n) -> p k n", n=CW)
            cn_ext3 = cn_ext[:, :].rearrange("p (k n) -> p k n", n=CW)

            # lead-in: contiguous pieces only (strided sub-chunks transfer
            # at a fraction of the 360 GB/s aggregate rate), in exact matmul
            # consumption order: the first row tile's stationary blocks,
            # then chunk 0's four K-slices, the remaining stationaries, and
            # the other column chunks as whole 1 MB pieces.
            nc.sync.dma_start(out=xw[:, : 2 * 2 * P], in_=xw_ext[:, : 2 * 2 * P])
            for kk in range(KC):
                nc.sync.dma_start(
                    out=cnt3[:, kk : kk + 1, :], in_=cn_ext3[:, kk : kk + 1, :]
                )
            nc.sync.dma_start(out=xw[:, 2 * 2 * P :], in_=xw_ext[:, 2 * 2 * P :])
            for c in range(1, NC_CH):
                nc.sync.dma_start(
                    out=cnt3[:, c * KC : (c + 1) * KC, :],
                    in_=cn_ext3[:, c * KC : (c + 1) * KC, :],
                )

            for c in range(NC_CH):
                for m in range(NM):
                    ps = psum.tile([P, CW], F32, name="ps")
                    for kg in range(NKG):
                        wo = (m * NKG + kg) * 2 * P
                        for h in range(NH):
                            nc.tensor.matmul(
                                out=ps[:, h * 512 : (h + 1) * 512],
                                lhsT=xw[:, wo : wo + 2 * P],
                                rhs=cnt3[
                                    :,
                                    c * KC + 2 * kg : c * KC + 2 * kg + 2,
                                    h * 512 : (h + 1) * 512,
                                ],
                                start=(kg == 0),
                                stop=(kg == NKG - 1),
                                perf_mode=mybir.MatmulPerfMode.DoubleRowSwInterleave,
                            )
                    # exp in place (PSUM out has lower access latency than
                    # SBUF and the exp values themselves are dead - only the
                    # accumulator row-sum is used)
                    nc.scalar.activation(
                        out=ps[:],
                        in_=ps[:],
                        func=mybir.ActivationFunctionType.Exp,
                        scale=EXP_SCALE,
                        accum_out=seacc[:, m * NC_CH + c : m * NC_CH + c + 1],
                    )
                    if c == NC_CH - 1:
                        # last column chunk for this row tile: its four
                        # accumulators are final - stream them out now so
                        # only the last 2KB DMA trails the final exp
                        nc.sync.dma_start(
                            out=out_ext[:, m * NC_CH : (m + 1) * NC_CH],
                            in_=seacc[:, m * NC_CH : (m + 1) * NC_CH],
                        )

    nc.compile()
    return nc


def _pack_stationary(xn8_rows):
    """[1024, 512] fp8 -> [128, NM*NKG*256] SwInterleave stationary blocks.

    Block (m, kg) at column offset (m*NKG+kg)*256 holds
    packed[p, 2*(127-r) + i] = xn8[m*128 + r, (2*kg+i)*128 + p].
    """
    a = xn8_rows.reshape(NM, P, NKG, 2, P)          # [m, r, kg, i, p]
    a = a.transpose(4, 0, 2, 1, 3)[:, :, :, ::-1, :]  # [p, m, kg, r(rev), i]
    return np.ascontiguousarray(a.reshape(P, NM * NKG * 2 * P))


def _pack_streaming(cn8):
    """[8192, 512] fp8 -> [128, NC_CH*KC*CW]: piece (c, kk) holds
    cn8.T[kk*128 + p, c*CW + n]."""
    a = cn8.T.reshape(KC, P, NC_CH, CW).transpose(1, 2, 0, 3)  # [p, c, kk, n]
    return np.ascontiguousarray(a.reshape(P, NC_CH * KC * CW))


def prepare(x, labels, W):
    """All host-side math: normalize, pack fp8 inputs, and return the
    per-row constants needed to assemble the loss from device row-sums."""
    x = np.asarray(x, dtype=np.float32)
    W = np.asarray(W, dtype=np.float32)
    labels = np.asarray(labels).astype(np.int64)

    centers = W[labels]                                  # [B, D]
    cn = centers / np.linalg.norm(centers, axis=1, keepdims=True)
    xn = x / np.maximum(np.linalg.norm(x, axis=1, keepdims=True), 1e-12)

    xn8 = (xn * GAM).astype(ml_dtypes.float8_e4m3)
    cn8 = (cn * GAM).astype(ml_dtypes.float8_e4m3)

    cnt = _pack_streaming(cn8)
    in_maps = []
    for k in range(NCORES):
        xw = _pack_stationary(xn8[k * BL : (k + 1) * BL])
        in_maps.append({"xw8": xw, "cnt8": cnt})

    # exact per-row scalars in f64
    xn64 = xn.astype(np.float64)
    cn64 = cn.astype(np.float64)
    t = np.clip(np.sum(xn64 * cn64, axis=1), -1.0, 1.0)
    tp = np.cos(np.arccos(t) + MARGIN)
    ecorr = np.exp(S_SCALE * tp) - np.exp(S_SCALE * t)
    rowlin = xn64 @ cn64.sum(axis=0) + (tp - t)          # sum_j cos'_ij
    return in_maps, t, tp, ecorr, rowlin


_compiled_nc = None


def get_compiled():
    global _compiled_nc
    if _compiled_nc is None:
        _compiled_nc = build_nc()
    return _compiled_nc


def run(x, labels, W, trace=False, trace_cores=None):
    nc = get_compiled()
    in_maps, t, tp, ecorr, rowlin = prepare(x, labels, W)
    res = run_bass_kernel_spmd(
        nc,
        in_maps,
        core_ids=list(range(NCORES)),
        trace=trace,
        trace_cores=trace_cores,
    )
    # sout[p, m*NC_CH + c] holds the partial sum over column chunk c for
    # local row m*128 + p; sum chunks, then flatten [m, p] -> local rows
    rowsum = np.concatenate(
        [
            np.asarray(r["sout"], dtype=np.float64)
            .reshape(P, NM, NC_CH)
            .sum(axis=2)
            .T.reshape(BL)
            for r in res.results
        ]
    )
    lse = np.log(rowsum + ecorr)
    a1 = (1.0 - EPS) + EPS * B / C
    loss = np.mean(
        a1 * lse - (1.0 - EPS) * S_SCALE * tp - (EPS / C) * S_SCALE * rowlin
    )
    return np.float32(loss), res


def kernel(**inputs):
    loss, _ = run(inputs["x"], inputs["labels"], inputs["W"])
    return loss
